# revision 38
# baseline (speedup 1.0000x reference)
"""Self-contained TRN2 Bass kernel for nn_MultiHeadAttn_91010357002583.

Multi-head attention (B=4, S=2048, D=1024, H=16, hd=64), eval mode,
mask all-ones, char_ids/seq_len unused by the reference.

Sharding: 8 cores = 4 batches x 2 query-row halves. Each core:
  - receives x^T (bf16) for its batch with ITS query half's rows FIRST
    (attention is permutation-invariant over key rows, so reordering
    k rows is free; q rows stay in original order within the half);
  - computes full K^T / V for the batch (2x redundant) + Q^T for its half;
  - attention with scores transposed [k, q]; V is stored packed per
    (ktile, head-pair) as [V_even 64 | ones 64 | V_odd 64] so each head's
    attnV stationary is one contiguous [128,128] slice that yields the
    weighted values on one 64-row half of psum and 64 copies of the
    softmax denominator on the other — one matmul, one accumulation
    group, full-width stationary;
  - per-head normalization: a partition-shifting reciprocal moves the
    denominator onto the numerator's base partitions (DVE tensor_tensor
    needs equal SB input bases), then one multiply; fc on its disjoint
    1024 output rows (y in bf16, converted to f32 on the host).
Output is a pure concatenation — no collectives, no host reduction.

Schedule: 256 steps of (scores -> exp -> attnV), one head at a time, with
heads rotated so pair (0,1) finishes last. attnV trails the exp stream by
two steps and the deadline-paced projection/fc slices sit between scores
and attnV in the in-order PE queue, so the PE never waits on the
activation. PSUM: score psum double-buffered (4 banks), one attnV
accumulator (2), a pool for projection/fc chains (2). fc accumulates j
blocks in head-completion order; most chains are paced into the late loop
steps (forcing exact semaphore waits — the Tile scheduler hoists a psum
chain's waits onto its first ldweights), and the final head pair's block
is folded in afterwards as a 1-matmul chainlet + DVE add.
"""

import math
import sys
from collections import deque
from contextlib import ExitStack

import numpy as np
import ml_dtypes

for _p in ("/opt/trn_rl_repo", "/root/.axon_site/_ro/trn_rl_repo"):
    if _p not in sys.path:
        sys.path.insert(0, _p)

import concourse.bass as bass  # noqa: E402
import concourse.tile as tile  # noqa: E402
from concourse import bacc, mybir  # noqa: E402
from concourse.bass_utils import run_bass_kernel_spmd  # noqa: E402

bf16 = ml_dtypes.bfloat16
FP32 = mybir.dt.float32
BF16 = mybir.dt.bfloat16
AF = mybir.ActivationFunctionType

B, S, D, H = 4, 2048, 1024, 16
HD = D // H
SCALE = math.sqrt(HD)


class Cfg:
    def __init__(self, R=2048, Q=1024, Hn=16, D=1024, repeats=1,
                 abufs=4, trail=2, half_kv=False, attnv_fake=False,
                 v64=False, av_mode="dual",
                 skip_units=False, skip_attnv=False, skip_exp=False):
        assert R % 128 == 0 and Q % 128 == 0 and Hn % 4 == 0
        self.R, self.Q, self.Hn, self.D = R, Q, Hn, D
        self.FT = D // 128          # feature tiles (proj contraction)
        self.NCT = Hn // 2          # coltiles for Q (and K) = heads/2
        self.NRT = R // 128         # k row tiles
        self.NJ = Hn * 64 // 128    # d-tiles for fc contraction
        self.NQT = Q // 128
        self.repeats = repeats
        self.abufs = abufs
        self.half_kv = half_kv      # timing proxy: only own half of K/V
        self.attnv_fake = attnv_fake  # timing: attnV reads a constant tile
        self.av_mode = av_mode      # attnV mode knob (packed is default)
        self.trail = trail          # attnV trails exp by this many steps
        self.v64 = v64              # timing: 64-wide attnV stationary
        self.skip_units = skip_units
        self.skip_attnv = skip_attnv
        self.skip_exp = skip_exp
        self.scale = 1.0 / math.sqrt(64.0)


def build_nc(cfg: Cfg, num_devices=8):
    R, Hn, Dm, FT = cfg.R, cfg.Hn, cfg.D, cfg.FT
    nc = bacc.Bacc("TRN2", target_bir_lowering=False, debug=False,
                   enable_asserts=False, num_devices=num_devices)
    xt_d = nc.dram_tensor("xt", [Dm, R], BF16, kind="ExternalInput").ap()
    wqk_d = nc.dram_tensor("wqk", [Hn, 128, FT, 128], BF16,
                           kind="ExternalInput").ap()
    wv_d = nc.dram_tensor("wv", [FT, 128, Hn * 64], BF16,
                          kind="ExternalInput").ap()
    wfc_d = nc.dram_tensor("wfc", [cfg.NJ, 128, Dm], BF16,
                           kind="ExternalInput").ap()
    bfc_d = nc.dram_tensor("bfc", [128, Dm], FP32, kind="ExternalInput").ap()
    y_d = nc.dram_tensor("y", [cfg.Q, Dm], BF16,
                     kind="ExternalOutput").ap()
    with tile.TileContext(nc) as tc:
        with ExitStack() as ctx:
            build_body(ctx, tc, cfg, xt_d, wqk_d, wv_d, wfc_d, bfc_d, y_d)
    nc.finalize()
    return nc


def build_body(ctx, tc, cfg: Cfg, xt_d, wqk_d, wv_d, wfc_d, bfc_d, y_d):
    nc = tc.nc
    R, Q, Hn, Dm, FT = cfg.R, cfg.Q, cfg.Hn, cfg.D, cfg.FT
    NCT, NRT, NJ, NQT = cfg.NCT, cfg.NRT, cfg.NJ, cfg.NQT

    persist = ctx.enter_context(tc.tile_pool(name="persist", bufs=1))
    wqk_pool = ctx.enter_context(tc.tile_pool(name="wqk", bufs=4))
    wv_pool = ctx.enter_context(tc.tile_pool(name="wv", bufs=2))
    attn_pool = ctx.enter_context(tc.tile_pool(name="attn", bufs=cfg.abufs))
    ysb_pool = ctx.enter_context(tc.tile_pool(name="ysb", bufs=6))
    den_pool = ctx.enter_context(tc.tile_pool(name="den", bufs=1))
    spool = ctx.enter_context(tc.tile_pool(name="ps_s", bufs=2, space="PSUM"))
    apool = ctx.enter_context(tc.tile_pool(name="ps_o", bufs=1, space="PSUM"))
    qpool = ctx.enter_context(tc.tile_pool(name="ps_q", bufs=2, space="PSUM"))

    for _rep in range(cfg.repeats):
        xt_sb = persist.tile([128, FT, R], BF16, tag="xt")
        KT_sb = persist.tile([128, NCT, R], BF16, tag="kt")
        QT_sb = persist.tile([128, NCT, Q], BF16, tag="qt")
        # V and a shared all-ones block live in ONE tensor so attnV can use
        # a two-group strided stationary AP [V(kt,h) | ones] — one 128-wide
        # matmul computes both the weighted values (out rows 0:64) and the
        # softmax denominator (rows 64:128) in a single accumulation group
        # packed V: per (kt, ct) 192 cols = [V_even 64 | ones 64 | V_odd 64]
        # so each head's attnV stationary is a PLAIN contiguous [128, 128]
        # slice (even: cols 0:128 -> rows 0:64 values, 64:128 denominator;
        # odd: cols 64:192 -> rows 0:64 denominator, 64:128 values) — one
        # matmul, one accumulation group, full-width stationary
        vpack = persist.tile([128, NRT, NCT, 192], BF16, tag="v")
        OT_js = [persist.tile([128, Q], BF16, tag=f"ot{j}", name=f"ot{j}")
                 for j in range(NJ)]
        wfc_sb = persist.tile([128, NJ, Dm], BF16, tag="wfc")
        bfc_sb = persist.tile([128, Dm], FP32, tag="bfc")
        if cfg.skip_units:
            # timing-only ablation: fabricate projection outputs
            nc.vector.memset(KT_sb[:], 0.01)
            nc.vector.memset(QT_sb[:], 0.01)
            nc.vector.memset(vpack[:], 0.01)
        elif cfg.half_kv:
            # timing proxy for the core-pair K/V exchange: other half of
            # K/V is fabricated, own half computed (same PE work as after
            # a real exchange)
            nc.vector.memset(KT_sb[:, :, R // 2:], 0.01)
            nc.vector.memset(vpack[:, NRT // 2:, :, :], 0.01)
        # the shared ones blocks (after the ablation memsets)
        nc.vector.memset(vpack[:, :, :, 64:128], 1.0)


        # batched xt loads: one DMA per quarter; quarter 0 is issued before
        # the prologue chains (whose weight DMAs follow immediately), the
        # rest after, so the first matmul starts ~5us in
        xt_dr = xt_d.rearrange("(f p) r -> p f r", p=128)

        def load_xt_quarter(quarter, per_ft=False):
            # gpsimd-triggered queue: runs parallel to the sync queue that
            # carries the weight loads, so the first chains start sooner
            if per_ft:
                for ft in range(FT):
                    nc.gpsimd.dma_start(
                        xt_sb[:, ft, quarter * 512:(quarter + 1) * 512],
                        xt_dr[:, ft, quarter * 512:(quarter + 1) * 512])
            else:
                nc.gpsimd.dma_start(
                    xt_sb[:, :, quarter * 512:(quarter + 1) * 512],
                    xt_dr[:, :, quarter * 512:(quarter + 1) * 512])

        preload_wqk = []

        wqk_tiles = {}

        def load_wqk(j):
            if j in wqk_tiles:
                return
            t = wqk_pool.tile([128, FT, 128], BF16, tag="wqk", name="wqk_t")
            nc.sync.dma_start(t[:], wqk_d[j])
            wqk_tiles[j] = t

        wv_tiles = {}

        def load_wv(g):
            if g in wv_tiles:
                return
            t = wv_pool.tile([128, FT, 256], BF16, tag="wv", name="wv_t")
            nc.sync.dma_start(
                t[:], wv_d[:, :, g * 256:(g + 1) * 256].rearrange(
                    "f p c -> p f c"))
            wv_tiles[g] = t

        # projection chains as generators: yield after each contraction
        # slice so the pacer can interleave a few matmuls per attention
        # step instead of parking a 16-matmul block in the PE queue
        def q_chain(ct):
            def emit():
                load_wqk(ct)
                ps = [qpool.tile([128, 512], FP32, tag="ps_q",
                                 name="ps_qc") for _ in range(2)]
                for c in range(2):
                    for ft in range(FT):
                        nc.tensor.matmul(
                            ps[c][:], wqk_tiles[ct][:, ft, :],
                            xt_sb[:, ft, c * 512:(c + 1) * 512],
                            start=(ft == 0), stop=(ft == FT - 1))
                        if ft % 2 == 1:
                            yield
                    nc.vector.tensor_copy(
                        QT_sb[:, ct, c * 512:(c + 1) * 512], ps[c][:])
            return emit

        def k_chain(ct, rh):
            def emit():
                load_wqk(NCT + ct)
                ps = [qpool.tile([128, 512], FP32, tag="ps_q",
                                 name="ps_kc") for _ in range(2)]
                for c in range(2):
                    for ft in range(FT):
                        nc.tensor.matmul(
                            ps[c][:], wqk_tiles[NCT + ct][:, ft, :],
                            xt_sb[:, ft, (2 * rh + c) * 512:
                                  (2 * rh + c + 1) * 512],
                            start=(ft == 0), stop=(ft == FT - 1))
                        if ft % 2 == 1:
                            yield
                    nc.vector.tensor_copy(
                        KT_sb[:, ct, (2 * rh + c) * 512:
                              (2 * rh + c + 1) * 512], ps[c][:])
            return emit

        def v_chain(rt, g):
            def emit():
                load_wv(g)
                ps = qpool.tile([128, 512], FP32, tag="ps_q",
                                name="ps_vc")[:, 0:256]
                for ft in range(FT):
                    nc.tensor.matmul(
                        ps[:], xt_sb[:, ft, rt * 128:(rt + 1) * 128],
                        wv_tiles[g][:, ft, :],
                        start=(ft == 0), stop=(ft == FT - 1))
                    if ft % 2 == 1:
                        yield
                pv = ps.rearrange("p (u q c) -> p u q c", u=2, q=2, c=64)
                nc.vector.tensor_copy(
                    vpack[:, rt, 2 * g:2 * g + 2, 0:64], pv[:, :, 0, :])
                nc.vector.tensor_copy(
                    vpack[:, rt, 2 * g:2 * g + 2, 128:192], pv[:, :, 1, :])
            return emit

        # heads are processed in rotated order so the pair (0,1) — fc's
        # last-accumulated j block — finishes last and its norm hides
        # under the other fc slices
        H_ORDER = list(range(2, Hn)) + [0, 1]
        slot = {h: i for i, h in enumerate(H_ORDER)}

        # ---- fc, chain-split ---- Tile hoists a psum chain's waits onto
        # its first ldweights, so a chain containing the last head pair's
        # j-block would stall until the final norm (~4.5us after the last
        # attnV). fc therefore accumulates the other NJ-1 blocks per qt
        # (most chains paced INTO the late loop steps, before the final
        # norm exists in program order, forcing exact waits), and the last
        # block is folded in afterwards via a 1-matmul chainlet + DVE add.
        j_order = sorted(range(NJ),
                         key=lambda j: max(slot[2 * j], slot[2 * j + 1]))
        j_order_main, j_last = j_order[:-1], j_order[-1]
        NSPLIT = 4
        yts = {}

        def fc_main(qt, js):
            yt = yts.setdefault(qt, ysb_pool.tile([128, Dm], BF16, tag="y",
                                                  name="yt"))
            for ch in range(2):
                ps = qpool.tile([128, 512], FP32, tag="ps_q", name="ps_fc")
                for ji, j in enumerate(js):
                    nc.tensor.matmul(
                        ps[:], OT_js[j][:, qt * 128:(qt + 1) * 128],
                        wfc_sb[:, j, ch * 512:(ch + 1) * 512],
                        start=(ji == 0), stop=(ji == len(js) - 1))
                nc.vector.tensor_add(yt[:, ch * 512:(ch + 1) * 512], ps[:],
                                     bfc_sb[:, ch * 512:(ch + 1) * 512])

        def fc_main_unit(qt):
            def emit():
                yt = yts.setdefault(qt, ysb_pool.tile(
                    [128, Dm], BF16, tag="y", name="yt"))
                for ch in range(2):
                    ps = qpool.tile([128, 512], FP32, tag="ps_q",
                                    name="ps_fc")
                    for ji, j in enumerate(j_order_main):
                        nc.tensor.matmul(
                            ps[:], OT_js[j][:, qt * 128:(qt + 1) * 128],
                            wfc_sb[:, j, ch * 512:(ch + 1) * 512],
                            start=(ji == 0),
                            stop=(ji == len(j_order_main) - 1))
                        if ji % 2 == 1:
                            yield
                    nc.vector.tensor_add(
                        yt[:, ch * 512:(ch + 1) * 512], ps[:],
                        bfc_sb[:, ch * 512:(ch + 1) * 512])
                    yield
            return emit

        def fc_last(qt):
            yt = yts[qt]
            for ch in range(2):
                ps = qpool.tile([128, 512], FP32, tag="ps_q", name="ps_fl")
                nc.tensor.matmul(
                    ps[:], OT_js[j_last][:, qt * 128:(qt + 1) * 128],
                    wfc_sb[:, j_last, ch * 512:(ch + 1) * 512],
                    start=True, stop=True)
                with nc.allow_low_precision(reason="y output is bf16"):
                    nc.vector.tensor_add(yt[:, ch * 512:(ch + 1) * 512],
                                         yt[:, ch * 512:(ch + 1) * 512],
                                         ps[:])
                nc.sync.dma_start(
                    y_d[qt * 128:(qt + 1) * 128, ch * 512:(ch + 1) * 512],
                    yt[:, ch * 512:(ch + 1) * 512])

        # unit list: (deadline, release, gen). deadline = step by which the
        # unit must be done (forced emission); release = step before which
        # the pacer must NOT pull it (so its reads exist in program order)
        units = []
        for ct in range(NCT):
            dl = NRT * min(slot[2 * ct], slot[2 * ct + 1])
            units.append((dl, 0, q_chain(ct)))
            if cfg.half_kv:
                units.append((dl, 0, k_chain(ct, 0)))
            else:
                for rh in range(2):
                    units.append((dl + 8 * rh, 0, k_chain(ct, rh)))
        v_rts = range(NRT // 2 if cfg.half_kv else NRT)
        for g in range(4):
            gdl = NRT * min(slot[h] for h in range(4 * g, 4 * g + 4))
            for rt in v_rts:
                units.append((gdl + rt, 0, v_chain(rt, g)))
        if not cfg.skip_attnv:
            # release 226: every j in j_order_main is normed by step 225
            for qt in range(NSPLIT):
                units.append((230 + 4 * qt, 226, fc_main_unit(qt)))
        units.sort(key=lambda d: d[0])

        if cfg.skip_units:
            units = []
        gq = deque(units)
        gstate = {"gen": None, "dl": 0}

        def adv(budget, S):
            while True:
                if gstate["gen"] is None:
                    if not gq:
                        return
                    if gq[0][1] > S:
                        return
                    gstate["dl"], _rel, gf = gq.popleft()
                    gstate["gen"] = gf()
                if budget <= 0 and gstate["dl"] > S + 2:
                    return
                try:
                    next(gstate["gen"])
                    budget -= 1
                except StopIteration:
                    gstate["gen"] = None

        fct = H_ORDER[0] // 2
        load_wqk(fct)
        load_wqk(NCT + fct)
        load_xt_quarter(0, per_ft=True)
        load_xt_quarter(1)
        adv(0, 0)  # deadline-0 units fully emitted before step 0
        for quarter in range(2, 4):
            load_xt_quarter(quarter)
        nc.sync.dma_start(wfc_sb[:], wfc_d.rearrange("j p d -> p j d"))
        nc.sync.dma_start(bfc_sb[:], bfc_d[:])

        NSTEP = Hn * NRT
        oA_cur = {}
        pend = deque()  # (h, kt, aA) whose attnV is not yet emitted

        fake_aA = None
        if cfg.skip_exp or cfg.attnv_fake:
            # timing ablation: attnV consumes a constant tile so the ACT
            # engine goes fully idle without dangling-read tiles
            fake_aA = persist.tile([128, Q], BF16, tag="fakeA")
            nc.vector.memset(fake_aA[:], 0.01)
        if cfg.skip_attnv:
            # fc still reads the OT tiles
            for j in range(NJ):
                nc.vector.memset(OT_js[j][:], 0.01)

        def emit_attnv(h, kt, aA):
            if cfg.skip_attnv:
                return
            if kt == 0:
                oA_cur[h] = apool.tile([128, 1024], FP32, tag="oA",
                                       name="oA")[:, :Q]
            oA = oA_cur[h]
            st, sp = (kt == 0), (kt == NRT - 1)
            src_a = fake_aA if cfg.attnv_fake else aA
            hct, par = h // 2, h % 2
            voff = par * 64  # even: [V|ones] cols 0:128, odd: [ones|V] 64:192
            if cfg.v64:
                sta = vpack[:, kt, hct, 2 * par * 64:2 * par * 64 + 64]
            else:
                sta = vpack[:, kt, hct, voff:voff + 128]
            for sc in range(0, Q, 512):
                sn = min(512, Q - sc)
                s_ = slice(sc, sc + sn)
                nc.tensor.matmul(oA[0:64 if cfg.v64 else 128, s_], sta,
                                 src_a[:, s_], start=st, stop=sp)

        def emit_norm(h):
            if cfg.skip_attnv:
                return
            ct, half = h // 2, h % 2
            oA = oA_cur.pop(h)
            # copy out fast to release the psum accumulator; for even heads
            # rows 0:64 are values and 64:128 denominators, odd swapped
            nrows = slice(0, 64) if half == 0 else slice(64, 128)
            drows = slice(64, 128) if half == 0 else slice(0, 64)
            stt = den_pool.tile([128, 1024], FP32, tag="stc",
                                name="stc")[:, :Q]
            nc.vector.tensor_copy(stt, oA[:, :])
            # reciprocal shifts the denominator onto the numerator's base
            # partitions (DVE tensor_tensor needs equal SB input bases)
            rden = den_pool.tile([128, 1024], BF16, tag="rden",
                                 name="rden")[:, :Q]
            with nc.allow_low_precision(reason="bf16 reciprocal of denom"):
                nc.vector.reciprocal(rden[nrows, :], stt[drows, :])
            nc.vector.tensor_mul(OT_js[ct][half * 64:half * 64 + 64, :],
                                 stt[nrows, :], rden[nrows, :])

        for St in range(NSTEP):
            hs, kt = divmod(St, NRT)
            h = H_ORDER[hs]
            ct, half = h // 2, h % 2
            rows = slice(64 * half, 64 * half + 64)
            psA = spool.tile([128, 1024], FP32, tag="ps_s",
                             name="psA")[:, :Q]
            for sc in range(0, Q, 512):
                sn = min(512, Q - sc)
                nc.tensor.matmul(
                    psA[:, sc:sc + sn],
                    KT_sb[rows, ct, kt * 128:(kt + 1) * 128],
                    QT_sb[rows, ct, sc:sc + sn], start=True, stop=True)
            if cfg.skip_exp:
                aA = fake_aA
            else:
                aA = attn_pool.tile([128, Q], BF16, tag="aT", name="aA")
                nc.scalar.activation(aA[:], psA, AF.Exp, scale=cfg.scale)
            # paced projection/fc slices sit BETWEEN scores(S) and the
            # trailing attnV in the PE queue, filling the exp window
            adv(2, St)
            # software pipeline: attnV trails the exp stream by TWO steps
            # so the in-order PE queue never waits on the activation
            pend.append((h, kt, aA))
            if len(pend) > cfg.trail:
                ph, pkt, paA = pend.popleft()
                emit_attnv(ph, pkt, paA)
                if pkt == NRT - 1:
                    emit_norm(ph)
        adv(10 ** 9, NSTEP)  # flush any remaining projection work
        while pend:
            ph, pkt, paA = pend.popleft()
            emit_attnv(ph, pkt, paA)
            if pkt == NRT - 1:
                emit_norm(ph)

        # ---- fc tail ---- (chainlets interleaved so yt buffers recycle)
        start = 0 if cfg.skip_attnv else NSPLIT
        li = 0
        for qt in range(start, NQT):
            fc_main(qt, j_order_main)
            if li < qt - 1:
                fc_last(li)
                li += 1
        while li < NQT:
            fc_last(li)
            li += 1


# ---------------- host side ----------------

def prep_core_inputs(cfg: Cfg, xb_perm, W_qkv, W_fc, b_fc):
    """xb_perm: [R, D] f32, rows already permuted (this core's q rows first)."""
    Dm, Hn, FT, NCT, NJ = cfg.D, cfg.Hn, cfg.FT, cfg.NCT, cfg.NJ
    xt = np.ascontiguousarray(xb_perm.T).astype(bf16)
    Wq = W_qkv[:, :NCT * 128]
    Wk = W_qkv[:, Dm:Dm + NCT * 128]
    Wv = W_qkv[:, 2 * Dm:2 * Dm + Hn * 64]
    wq_t = Wq.reshape(FT, 128, NCT, 128).transpose(2, 1, 0, 3)
    wk_t = Wk.reshape(FT, 128, NCT, 128).transpose(2, 1, 0, 3)
    wqk = np.ascontiguousarray(
        np.concatenate([wq_t, wk_t], axis=0)).astype(bf16)
    wv = np.ascontiguousarray(Wv.reshape(FT, 128, Hn * 64)).astype(bf16)
    wfc = np.ascontiguousarray(
        W_fc[:NJ * 128].reshape(NJ, 128, Dm)).astype(bf16)
    bfc = np.ascontiguousarray(
        np.broadcast_to(b_fc.astype(np.float32), (128, Dm)))
    return {"xt": xt, "wqk": wqk, "wv": wv, "wfc": wfc, "bfc": bfc}


_CACHE = {}


def _get_nc(repeats=1, **kw):
    key = ("nc", repeats, tuple(sorted(kw.items())))
    if key not in _CACHE:
        _CACHE[key] = build_nc(Cfg(R=S, Q=S // 2, Hn=H, D=D,
                                   repeats=repeats, **kw))
    return _CACHE[key]


def make_in_maps(x, W_qkv, W_fc, b_fc):
    cfg = Cfg(R=S, Q=S // 2, Hn=H, D=D)
    x = np.asarray(x, dtype=np.float32)
    in_maps = []
    for c in range(8):
        b, half = divmod(c, 2)
        r0 = half * (S // 2)
        order = np.concatenate([
            np.arange(r0, r0 + S // 2),
            np.arange(0, r0),
            np.arange(r0 + S // 2, S),
        ])
        xb = x[b][order]
        in_maps.append(prep_core_inputs(
            cfg, xb, np.asarray(W_qkv, np.float32),
            np.asarray(W_fc, np.float32), np.asarray(b_fc, np.float32)))
    return in_maps


def kernel(x, char_ids, seq_len, mask, W_qkv, W_fc, b_fc):
    """Full inputs in, full [B, S, D] float32 output out."""
    import os
    # the axon NTFF trace hook is unavailable in this container; make sure
    # an inherited BASS_TRACE=1 cannot send us down that (crashing) path
    os.environ["BASS_NEVER_TRACE"] = "1"
    nc = _get_nc(repeats=1)
    in_maps = make_in_maps(x, W_qkv, W_fc, b_fc)
    res = run_bass_kernel_spmd(nc, in_maps, core_ids=list(range(8)))
    out = np.empty((B, S, D), dtype=np.float32)
    for c in range(8):
        b, half = divmod(c, 2)
        r0 = half * (S // 2)
        out[b, r0:r0 + S // 2, :] = res.results[c]["y"]
    return out
